# revision 27
# baseline (speedup 1.0000x reference)
"""DBHead (non-local attention + binarize/threshold conv branches) on 8 trn2 cores.

Sharding: 8 shards = 4 batch x 2 row-halves. Core (b, s) computes output rows
[128s, 128s+128) of the [3, 256, 256] map for batch b. All per-core variation
(which rows, halo padding, query-row masking) is pushed into host-prepared
input data so ONE SPMD program serves all 8 cores.

Attention layout: scores are built transposed (S^T: keys on partitions) so the
softmaxed tiles feed the S@V matmul directly as the moving operand; exp runs on
the scalar engine as the PSUM eviction; the softmax denominator is accumulated
on the vector engine and applied as a broadcast multiply.

Matmul dtypes: f16 operands everywhere (1 cycle/row, avoids the fp32_mode=HIGH
power throttle that capped PE utilization at 50%), except E/V in bf16 (exp of
unshifted scores reaches e^42 — needs bf16 range) and the tiny f32r softmax
rowsum/broadcast matmuls. PSUM accumulation is always f32. ConvT taps are
packed in pairs (M=128) to halve dw1/dw2 matmul rows.
"""
import sys, os
sys.path.insert(0, "/opt/trn_rl_repo")
import numpy as np
from contextlib import ExitStack

import concourse.bass as bass
import concourse.tile as tile
from concourse import mybir, bacc
from concourse.bass_utils import run_bass_kernel_spmd

F32 = mybir.dt.float32
F32R = mybir.dt.float32r
F16 = mybir.dt.float16
BF16 = mybir.dt.bfloat16
# matmul operand dtype for x/weights/conv stages (E/V always bf16)
XDT = BF16 if os.environ.get("KERNEL_BF16", "0") == "1" else F16
AFT = mybir.ActivationFunctionType
ALU = mybir.AluOpType

EPS = 1e-5
NQ = 2176  # 34 rows x 64 cols of query positions (33 real + 1 zero halo row)
QBLOCKS = [(0, 448), (448, 448), (896, 448), (1344, 448), (1792, 384)]
NKC = 32  # key chunks of 128 over 4096 positions

# wpack column offsets (f16, rows = contraction dim on partitions).
# Threshold-branch weights lead so its DMA chunk lands first and the branch
# can start while the rest of the inputs stream in.
OFF_THW = 0       # 18 x 64  (tap*2+chunk)
OFF_DW1TH = 1152  # 2 tap-pairs x 128 (rows 0:64)
OFF_DW2TH = 1408  # block-diag [128, 8]: rows 0:64 -> cols 0:4, rows 64:128 -> 4:8
W_TH_END = 1416
OFF_WQ = 1416     # 2 chunks x 64
OFF_WK = 1544
OFF_WA = 1672     # 2 chunks x 256
W_ATT_END = 2184
OFF_BZW = 2184
OFF_DW1BZ = 3336
OFF_DW2BZ = 3592
WCOLS = 3600

# bpack columns
BP_BQ, BP_BK = 0, 1
BP_BZ_S1, BP_BZ_B1, BP_BZ_S2, BP_BZ_B2 = 2, 3, 4, 5
BP_TH_S1, BP_TH_B1, BP_TH_S2, BP_TH_B2 = 6, 7, 8, 9
BP_BZ_DB2, BP_TH_DB2 = 10, 11
BCOLS = 16

_CACHE = {}
LAST_RESULTS = None


def _branch_ir(nc, tc, wr, bpk, hc, pcv, pct, ppt, pads, offw3, s1, b1,
               offdw1, s2, b2, offdw2, db2, out_dram, Tt=None, outB=None):
    """One conv branch. With Tt/outB set (binarize), the diff-binarize map is
    fused per output chunk so it overlaps the remaining matmuls."""
    AFT_ = AFT
    pts = {}
    for blk in range(4):
        cv = pcv.tile([64, 512], F32, tag="cv", name="cv")
        for t in range(9):
            ky, kx = t // 3, t % 3
            for c in range(2):
                o = offw3 + (t * 2 + c) * 64
                nc.tensor.matmul(
                    cv[:], lhsT=wr[:, o:o + 64],
                    rhs=pads[c][:, blk * 8 + ky:blk * 8 + ky + 8, kx:kx + 64],
                    start=(t == 0 and c == 0), stop=(t == 8 and c == 1))
        h1c = hc.tile([64, 512], XDT, tag="h1c", name="h1c")
        nc.scalar.activation(h1c[:], cv[:], AFT_.Relu,
                             bias=bpk[0:64, b1:b1 + 1],
                             scale=bpk[0:64, s1:s1 + 1])
        for tp in range(2):  # pair of convT1 taps packed on 128 partitions
            ct = pct.tile([128, 512], F32, tag="ct", name="ct")
            o = offdw1 + tp * 128
            nc.tensor.matmul(ct[:], lhsT=wr[0:64, o:o + 128], rhs=h1c[:],
                             start=True, stop=True)
            h2c = hc.tile([128, 512], XDT, tag="h2c", name="h2c")
            nc.scalar.activation(h2c[:], ct[:], AFT_.Relu,
                                 bias=bpk[0:128, b2:b2 + 1],
                                 scale=bpk[0:128, s2:s2 + 1])
            if blk % 2 == 0:
                pts[tp] = ppt.tile([8, 2, 512], F32, tag="pt", name="pt")
            pt = pts[tp]
            nc.tensor.matmul(pt[:, blk % 2, :], lhsT=wr[0:128, offdw2:offdw2 + 8],
                             rhs=h2c[:], start=True, stop=True)
            if blk % 2 == 1:
                win = slice((blk - 1) * 512, (blk + 1) * 512)
                ss = hc.tile([8, 2, 512], F32, tag="sg", name="sg")
                nc.scalar.activation(ss[:], pt[:], AFT_.Sigmoid,
                                     bias=bpk[0:8, db2:db2 + 1])
                nc.scalar.dma_start(out_dram[:, tp, win], ss[:])
                if Tt is not None:
                    dc = hc.tile([8, 2, 512], F32, tag="dc", name="dc")
                    eng = nc.vector if tp == 0 else nc.gpsimd
                    eng.tensor_sub(
                        dc[:].rearrange("p a c2 -> p (a c2)"),
                        ss[:].rearrange("p a c2 -> p (a c2)"), Tt[:, tp, win])
                    bb = hc.tile([8, 2, 512], F32, tag="bb", name="bb")
                    nc.scalar.activation(bb[:], dc[:], AFT_.Sigmoid, scale=50.0)
                    nc.gpsimd.dma_start(outB[:, tp, win], bb[:])


def _build():
    nc = bacc.Bacc("TRN2", target_bir_lowering=False, debug=False, num_devices=8)
    xin_d = nc.dram_tensor("xin", [256, 64, 64], XDT, kind="ExternalInput").ap()
    xq_d = nc.dram_tensor("xq", [256, 34, 64], XDT, kind="ExternalInput").ap()
    xpad_d = nc.dram_tensor("xpad", [256, 34, 66], XDT, kind="ExternalInput").ap()
    qm_d = nc.dram_tensor("qmask", [1, NQ], F32, kind="ExternalInput").ap()
    wp_d = nc.dram_tensor("wpack", [128, WCOLS], XDT, kind="ExternalInput").ap()
    bp_d = nc.dram_tensor("bpack", [128, BCOLS], F32, kind="ExternalInput").ap()
    ba_d = nc.dram_tensor("ba", [256], F32, kind="ExternalInput").ap()
    out_d = nc.dram_tensor("out", [3, 8, 2, 2048], F32, kind="ExternalOutput").ap()

    with tile.TileContext(nc) as tc, ExitStack() as ctx:
        cp = ctx.enter_context(tc.tile_pool(name="const", bufs=1))
        pp = ctx.enter_context(tc.tile_pool(name="pads", bufs=1))

        wr = cp.tile([128, WCOLS], XDT)
        bpk = cp.tile([128, BCOLS], F32)
        nc.scalar.dma_start(bpk[:], bp_d[:])
        nc.sync.dma_start(wr[:, 0:W_TH_END], wp_d[:, 0:W_TH_END])
        # threshold-branch inputs spread across all three DMA queues so the
        # PE has work as early as possible
        ba_f = cp.tile([1, 256], F32)
        qm = cp.tile([1, NQ], F32)
        nc.gpsimd.dma_start(ba_f[:], bass.AP(tensor=ba_d.tensor, offset=ba_d.offset,
                                             ap=[[0, 1]] + [list(a) for a in ba_d.ap]))
        nc.gpsimd.dma_start(qm[:], qm_d[:])
        bar = cp.tile([1, 256], F32R)
        nc.vector.tensor_copy(bar[:], ba_f[:])
        ones_f = cp.tile([128, 128], F32)
        nc.vector.memset(ones_f[:], 1.0)
        ones = cp.tile([128, 128], F32R)
        nc.vector.tensor_copy(ones[:], ones_f[:])
        zc = cp.tile([128, 34], F32)
        nc.vector.memset(zc[:], 0.0)

        xpr = [pp.tile([128, 34, 66], XDT, tag=f"xp{c}", name=f"xpr{c}")
               for c in range(2)]
        xnp = [pp.tile([128, 34, 66], XDT, tag=f"xn{c}", name=f"xnp{c}")
               for c in range(2)]
        nc.scalar.dma_start(xpr[0][:], xpad_d[0:128])
        nc.gpsimd.dma_start(xpr[1][:], xpad_d[128:256])
        nc.sync.dma_start(wr[:, W_TH_END:W_ATT_END], wp_d[:, W_TH_END:W_ATT_END])
        nc.sync.dma_start(wr[:, W_ATT_END:WCOLS], wp_d[:, W_ATT_END:WCOLS])
        Tt = cp.tile([8, 2, 2048], F32)
        baB = cp.tile([128, 256], F32)

        with tc.tile_pool(name="att", bufs=1) as ap_:
            e1r = ap_.tile([64, NQ], XDT)
            e2r = ap_.tile([64, 4096], XDT)
            V = ap_.tile([128, NKC, 256], BF16)

            with tc.tile_pool(name="xr", bufs=1) as xp:
                xr = [xp.tile([128, 64, 64], XDT, tag=f"xr{c}", name=f"xr{c}")
                      for c in range(2)]
                xqr = [xp.tile([128, 34, 64], XDT, tag=f"xq{c}", name=f"xqr{c}")
                       for c in range(2)]
                # chunked so early consumers start before full tiles land
                for c in range(2):
                    sl = slice(c * 128, (c + 1) * 128)
                    for r0, r1 in ((0, 17), (17, 34)):
                        nc.gpsimd.dma_start(xqr[c][:, r0:r1], xq_d[sl, r0:r1])
                for c in range(2):
                    sl = slice(c * 128, (c + 1) * 128)
                    for r0, r1 in ((0, 32), (32, 64)):
                        nc.scalar.dma_start(xr[c][:, r0:r1], xin_d[sl, r0:r1])

                # threshold branch: independent of attention; fills the PE
                # while attention inputs stream in
                with tc.tile_pool(name="hct", bufs=3) as hct, \
                     tc.tile_pool(name="pcv0", bufs=2, space="PSUM") as pcv0, \
                     tc.tile_pool(name="pct0", bufs=2, space="PSUM") as pct0, \
                     tc.tile_pool(name="ppt0", bufs=2, space="PSUM") as ppt0:
                    _branch_ir(nc, tc, wr, bpk, hct, pcv0, pct0, ppt0, xpr,
                               OFF_THW, BP_TH_S1, BP_TH_B1, OFF_DW1TH,
                               BP_TH_S2, BP_TH_B2, OFF_DW2TH, BP_TH_DB2,
                               out_d[1])
                # zero borders of xn pads (cols 0/65); rows are fully written later
                for c in range(2):
                    for col in (0, 65):
                        nc.vector.tensor_copy(
                            xnp[c][:, :, col:col + 1],
                            zc[:].rearrange("p (r o) -> p r o", o=1))

                xr_f = [t[:].rearrange("p r c2 -> p (r c2)") for t in xr]
                xq_f = [t[:].rearrange("p r c2 -> p (r c2)") for t in xqr]

                # ---- phase 1: e1 (queries), e2 (keys), V (values, pos-major) ----
                with tc.tile_pool(name="pe", bufs=2, space="PSUM") as pe, \
                     tc.tile_pool(name="pv", bufs=2, space="PSUM") as pv:
                    pb = pe.tile([128, 512], F32, tag="pe", name="pe_t")
                    nc.tensor.matmul(pb[:, 0:256], lhsT=ones[0:1, 0:128],
                                     rhs=bar[:], start=True, stop=True)
                    nc.vector.tensor_copy(baB[:], pb[:, 0:256])
                    for q0, w in QBLOCKS:
                        p = pe.tile([64, 512], F32, tag="pe", name="pe_t")
                        for c in range(2):
                            o = OFF_WQ + c * 64
                            nc.tensor.matmul(p[:, :w], lhsT=wr[:, o:o + 64],
                                             rhs=xq_f[c][:, q0:q0 + w],
                                             start=(c == 0), stop=(c == 1))
                        nc.scalar.activation(e1r[:, q0:q0 + w], p[:, :w], AFT.Prelu,
                                             bias=bpk[0:64, BP_BQ:BP_BQ + 1],
                                             alpha=0.25)
                    for k0 in range(0, 4096, 512):
                        p = pe.tile([64, 512], F32, tag="pe", name="pe_t")
                        for c in range(2):
                            o = OFF_WK + c * 64
                            nc.tensor.matmul(p[:], lhsT=wr[:, o:o + 64],
                                             rhs=xr_f[c][:, k0:k0 + 512],
                                             start=(c == 0), stop=(c == 1))
                        nc.scalar.activation(e2r[:, k0:k0 + 512], p[:], AFT.Prelu,
                                             bias=bpk[0:64, BP_BK:BP_BK + 1],
                                             alpha=0.25)
                    for jj in range(NKC // 2):
                        p = pv.tile([128, 2, 256], F32, tag="pv", name="pv_t")
                        for u in range(2):
                            j = 2 * jj + u
                            for c in range(2):
                                o = OFF_WA + c * 256
                                nc.tensor.matmul(p[:, u, :],
                                                 lhsT=xr_f[c][:, j * 128:(j + 1) * 128],
                                                 rhs=wr[:, o:o + 256],
                                                 start=(c == 0), stop=(c == 1))
                            nc.vector.tensor_add(p[:, u, :], p[:, u, :], baB[:])
                        nc.scalar.activation(V[:, 2 * jj:2 * jj + 2, :], p[:],
                                             AFT.Prelu, alpha=0.25)

            # ---- phase 2: attention blocks ----
            with tc.tile_pool(name="psc", bufs=3, space="PSUM") as psc, \
                 tc.tile_pool(name="pxn", bufs=4, space="PSUM") as pxn, \
                 tc.tile_pool(name="pr", bufs=1, space="PSUM") as pr, \
                 tc.tile_pool(name="eb", bufs=6) as eb, \
                 tc.tile_pool(name="rc", bufs=2) as rc:
                # softmax denominator chain for a finished block, staged
                # across the NEXT block's j-loop (different engines) so the
                # in-order tensor queue never stalls on it
                def tail_a(pend):
                    q0, w = pend["q0"], pend["w"]
                    # rs and rb have disjoint lifetimes: one shared PSUM bank
                    rs = pr.tile([128, 512], F32, tag="rr", name="rs")
                    nc.tensor.matmul(rs[0:1, :w], lhsT=ones[:, 0:1],
                                     rhs=pend["racc"][:, :w], start=True, stop=True)
                    rrow = rc.tile([1, 512], F32, tag="rrow", name="rrow")
                    nc.vector.reciprocal(rrow[:, :w], rs[0:1, :w])
                    rrm = rc.tile([1, 512], F32R, tag="rrm", name="rrm")
                    nc.gpsimd.tensor_mul(rrm[:, :w], rrow[:, :w], qm[:, q0:q0 + w])
                    pend["rrm"] = rrm

                def tail_b(pend):
                    w = pend["w"]
                    rb = pr.tile([128, 512], F32, tag="rr", name="rb")
                    nc.tensor.matmul(rb[:, :w], lhsT=ones[0:1, 0:128],
                                     rhs=pend["rrm"][:, :w], start=True, stop=True)
                    rbs = rc.tile([128, 512], F32, tag="rbs", name="rbs")
                    nc.vector.tensor_copy(rbs[:, :w], rb[:, :w])
                    pend["rbs"] = rbs

                def tail_c(pend):
                    w, rows, r0 = pend["w"], pend["rows"], pend["r0"]
                    for t in range(2):
                        nc.vector.tensor_mul(
                            xnp[t][:, r0:r0 + rows, 1:65],
                            pend["xn_ps"][t][:, :w].rearrange("p (r c2) -> p r c2", c2=64),
                            pend["rbs"][:, :w].rearrange("p (r c2) -> p r c2", c2=64))

                def emit_tail(pend):
                    tail_a(pend); tail_b(pend); tail_c(pend)

                pending = None
                for q0, w in QBLOCKS:
                    rows, r0 = w // 64, q0 // 64
                    xn_ps = [pxn.tile([128, 512], F32, tag="xn", name="xnps")
                             for _ in range(2)]
                    racc = rc.tile([128, 512], F32R, tag="racc", name="racc")
                    rac2 = rc.tile([128, 512], F32R, tag="rac2", name="rac2")
                    for j in range(NKC):
                        if pending is not None:
                            if j == 2:
                                tail_a(pending)
                            elif j == 6:
                                tail_b(pending)
                            elif j == 10:
                                tail_c(pending)
                                pending = None
                        sc = psc.tile([128, 512], F32, tag="sc", name="sc")
                        nc.tensor.matmul(sc[:, :w], lhsT=e2r[:, j * 128:(j + 1) * 128],
                                         rhs=e1r[:, q0:q0 + w], start=True, stop=True)
                        E = eb.tile([128, 512], BF16, tag="E", name="E")
                        nc.scalar.activation(E[:, :w], sc[:, :w], AFT.Exp)
                        for t in range(2):
                            nc.tensor.matmul(xn_ps[t][:, :w],
                                             lhsT=V[:, j, t * 128:(t + 1) * 128],
                                             rhs=E[:, :w],
                                             start=(j == 0), stop=(j == NKC - 1))
                        eng = nc.vector if j % 2 == 0 else nc.gpsimd
                        acc = racc if j % 2 == 0 else rac2
                        if j < 2:
                            eng.tensor_copy(acc[:, :w], E[:, :w])
                        else:
                            eng.tensor_add(acc[:, :w], acc[:, :w], E[:, :w])
                    nc.vector.tensor_add(racc[:, :w], racc[:, :w], rac2[:, :w])
                    pending = {"q0": q0, "w": w, "rows": rows, "r0": r0,
                               "xn_ps": xn_ps, "racc": racc}
                emit_tail(pending)

        # ---- phase 3: conv branches (att pool closed; SBUF freed) ----
        with tc.tile_pool(name="hc", bufs=3) as hc, \
             tc.tile_pool(name="pcv", bufs=2, space="PSUM") as pcv, \
             tc.tile_pool(name="pct", bufs=2, space="PSUM") as pct, \
             tc.tile_pool(name="ppt", bufs=2, space="PSUM") as ppt:
            nc.sync.dma_start(Tt[:], out_d[1])
            _branch_ir(nc, tc, wr, bpk, hc, pcv, pct, ppt, xnp, OFF_BZW,
                       BP_BZ_S1, BP_BZ_B1, OFF_DW1BZ, BP_BZ_S2, BP_BZ_B2,
                       OFF_DW2BZ, BP_BZ_DB2, out_d[0], Tt=Tt, outB=out_d[2])

    nc.compile()
    return nc


def _prep(inputs):
    """Host-side parameter prep shared by all cores (numpy, tiny)."""
    g = {k: np.asarray(v, np.float32) for k, v in inputs.items()}
    wpack = np.zeros((128, WCOLS), np.float32)
    wqT = g["wm1"].reshape(64, 256).T
    wpack[:, OFF_WQ:OFF_WQ + 64] = wqT[0:128]
    wpack[:, OFF_WQ + 64:OFF_WQ + 128] = wqT[128:256]
    wkT = g["wm2"].reshape(64, 256).T
    wpack[:, OFF_WK:OFF_WK + 64] = wkT[0:128]
    wpack[:, OFF_WK + 64:OFF_WK + 128] = wkT[128:256]
    waT = g["wa"].reshape(256, 256).T
    wpack[:, OFF_WA:OFF_WA + 256] = waT[0:128]
    wpack[:, OFF_WA + 256:OFF_WA + 512] = waT[128:256]
    for name, off in (("bz_cw", OFF_BZW), ("th_cw", OFF_THW)):
        w3 = g[name].transpose(2, 3, 1, 0).reshape(9, 256, 64)
        for t in range(9):
            for c in range(2):
                wpack[:, off + (t * 2 + c) * 64:off + (t * 2 + c) * 64 + 64] = \
                    w3[t, c * 128:(c + 1) * 128]
    # conv_transpose flips the kernel: tap (di,dj) uses w[1-di, 1-dj]
    # taps packed in pairs: pair tp holds taps 2tp (cols 0:64) and 2tp+1 (64:128)
    for name, off in (("bz_dw1", OFF_DW1BZ), ("th_dw1", OFF_DW1TH)):
        d1 = g[name].reshape(4, 64, 64)[::-1]
        for t in range(4):
            wpack[0:64, off + t * 64:off + (t + 1) * 64] = d1[t]
    # dw2 block-diagonal: rows 0:64 -> cols 0:4 (even tap's h2), 64:128 -> 4:8
    for name, off in (("bz_dw2", OFF_DW2BZ), ("th_dw2", OFF_DW2TH)):
        d2 = g[name].transpose(2, 0, 1, 3).reshape(64, 4)[:, ::-1]
        wpack[0:64, off:off + 4] = d2
        wpack[64:128, off + 4:off + 8] = d2

    bpack = np.zeros((128, BCOLS), np.float32)
    bpack[0:64, BP_BQ] = g["bm1"]
    bpack[0:64, BP_BK] = g["bm2"]
    for pre, (cs1, cb1, cs2, cb2, cdb2) in (
            ("bz", (BP_BZ_S1, BP_BZ_B1, BP_BZ_S2, BP_BZ_B2, BP_BZ_DB2)),
            ("th", (BP_TH_S1, BP_TH_B1, BP_TH_S2, BP_TH_B2, BP_TH_DB2))):
        inv1 = g[f"{pre}_g1"] / np.sqrt(g[f"{pre}_v1"] + EPS)
        bpack[0:64, cs1] = inv1
        bpack[0:64, cb1] = g[f"{pre}_b1"] - g[f"{pre}_m1"] * inv1
        inv2 = g[f"{pre}_g2"] / np.sqrt(g[f"{pre}_v2"] + EPS)
        bpack[0:64, cs2] = inv2
        bpack[64:128, cs2] = inv2
        b2v = g[f"{pre}_b2"] + (g[f"{pre}_db1"] - g[f"{pre}_m2"]) * inv2
        bpack[0:64, cb2] = b2v
        bpack[64:128, cb2] = b2v
        bpack[0:8, cdb2] = float(g[f"{pre}_db2"][0])
    import ml_dtypes
    xdt = ml_dtypes.bfloat16 if os.environ.get("KERNEL_BF16", "0") == "1" else np.float16
    return g, wpack.astype(xdt), bpack


def kernel(**inputs):
    global LAST_RESULTS
    if "nc" not in _CACHE:
        _CACHE["nc"] = _build()
    nc = _CACHE["nc"]
    g, wpack, bpack = _prep(inputs)
    x = g["x"]  # [4, 256, 64, 64]

    in_maps = []
    for core in range(8):
        b, s = core % 4, core // 4
        xq = np.zeros((256, 34, 64), np.float32)
        xpad = np.zeros((256, 34, 66), np.float32)
        qmask = np.ones((1, NQ), np.float32)
        if s == 0:
            xq[:, 1:34] = x[b][:, 0:33]
            xpad[:, 1:34, 1:65] = x[b][:, 0:33]
            qmask[0, 0:64] = 0.0
        else:
            xq[:, 0:33] = x[b][:, 31:64]
            xpad[:, 0:33, 1:65] = x[b][:, 31:64]
            qmask[0, 33 * 64:] = 0.0
        import ml_dtypes
        xdt = ml_dtypes.bfloat16 if os.environ.get("KERNEL_BF16", "0") == "1" else np.float16
        in_maps.append({"xin": np.ascontiguousarray(x[b]).astype(xdt),
                        "xq": xq.astype(xdt), "xpad": xpad.astype(xdt),
                        "qmask": qmask, "wpack": wpack, "bpack": bpack,
                        "ba": g["ba"]})

    br = run_bass_kernel_spmd(
        nc, in_maps, core_ids=list(range(8)),
        trace=os.environ.get("KERNEL_TRACE", "0") == "1")
    LAST_RESULTS = br

    out = np.zeros((4, 3, 256, 256), np.float32)
    for core in range(8):
        b, s = core % 4, core // 4
        raw = br.results[core]["out"].reshape(3, 2, 2, 2, 2, 32, 64)
        # [ch, half, ei, ej, tp, r, c]; dw1 tap t = 2*tp + half -> (a, b) =
        # (tp, half); output rows (r, a, ei), cols (c, b, ej)
        half = raw.transpose(0, 5, 4, 2, 6, 1, 3).reshape(3, 128, 256)
        out[b, :, 128 * s:128 * (s + 1), :] = half
    return out
